# revision 14
# baseline (speedup 1.0000x reference)
"""Trainium2 Bass kernel for an 8-batch decoder transformer layer.

Sharding: data-parallel over batch — core b computes batch element b
end-to-end (no collectives). Weights are replicated to all 8 cores.

Per-core math (S=1024, D=1024, H=16, DH=64, F=4096):
  sa  = causal MHA(x, x, x; wq1,wk1,wv1,wo1)
  x1  = LN(sa + x)
  ca  = MHA(x1, enc, enc; wq2,wk2,wv2,wo2)
  x2  = LN(ca + x1)
  ff  = gelu(x2 @ w1.T + b1) @ w2.T + b2
  out = LN(ff + x2)

Layout strategy:
  - activations kept transposed [D, S] for matmul operands (contraction on
    partitions); attention scores computed transposed [S_k, S_q] so the A@V
    matmul needs no on-chip transposes of the attention matrix.
  - softmax denominators via a ones-column appended to V (lhsT M=65).
  - causal mask added into the scores PSUM with an identity-matmul
    accumulate of a -1e6 addend tile, before a single exp() pass.
  - fp32r matmuls for projections/scores; bf16 for exp(scores), V, heads,
    and the WO/FFN matmuls.  LayerNorm in natural [S, D] layout via
    bn_stats/bn_aggr; activations re-transposed between phases on the PE.
"""

import os
import sys
from contextlib import ExitStack

import numpy as np

for _p in ("/opt/trn_rl_repo", "/root/.axon_site/_ro/trn_rl_repo"):
    if os.path.isdir(_p) and _p not in sys.path:
        sys.path.append(_p)

import concourse.bass as bass  # noqa: E402
from concourse import bacc  # noqa: E402
import concourse.tile as tile  # noqa: E402
from concourse import mybir  # noqa: E402
from concourse.bass_utils import run_bass_kernel_spmd  # noqa: E402

F32 = mybir.dt.float32
F32R = mybir.dt.float32r
BF16 = mybir.dt.bfloat16
AF = mybir.ActivationFunctionType
ALU = mybir.AluOpType

B, S, D, H, F = 8, 1024, 1024, 16, 4096
DH = D // H          # 64
P = 128
DC = D // P          # 8 contraction chunks
SC = S // P          # 8 sequence tiles
NPAIR = H // 2       # 8 head pairs
FC = F // P          # 32 f-tiles
EPS = 1e-5
NEG = -1.0e6


def _r(ap):
    return ap.bitcast(F32R)


def _ln(nc, stats_pool, r_t, out_t, eps_t, g_t=None, b_t=None):
    """LayerNorm along the free dim of r_t [128, 1024] -> out_t."""
    stats = stats_pool.tile([P, 2, 6], F32, tag="ln_stats", name="ln_stats")
    nc.vector.bn_stats(out=stats[:, 0, :], in_=r_t[:, 0:512])
    nc.vector.bn_stats(out=stats[:, 1, :], in_=r_t[:, 512:1024])
    mv = stats_pool.tile([P, 2], F32, tag="ln_mv", name="ln_mv")
    nc.vector.bn_aggr(out=mv[:], in_=stats[:])
    nc.scalar.activation(mv[:, 1:2], mv[:, 1:2], AF.Sqrt, bias=eps_t[:])
    nc.vector.reciprocal(mv[:, 1:2], mv[:, 1:2])
    nc.vector.tensor_scalar(
        out=out_t,
        in0=r_t,
        scalar1=mv[:, 0:1],
        scalar2=mv[:, 1:2],
        op0=ALU.subtract,
        op1=ALU.mult,
    )
    if g_t is not None:
        nc.vector.tensor_tensor(out_t, out_t, g_t, ALU.mult)
    if b_t is not None:
        nc.vector.tensor_tensor(out_t, out_t, b_t, ALU.add)


def _projection_T(ctx, tc, dst, src_T, w_dram, pname, bf=False):
    """dst[:, p, :] (f32 [128, NPAIR, S]) = (x @ W)^T for head-pair p.

    src_T: [128, DC, S] (x transposed); w_dram: [D, D] (W^T layout,
    col h*DH+k = head h dim k).  lhsT = W^T column block, rhs = src_T.
    bf=True: bf16 operands (src_T and w_dram must be bf16).
    """
    nc = tc.nc
    wdt = BF16 if bf else F32R
    cast = lambda a: a
    wp = ctx.enter_context(tc.tile_pool(name=f"wp_{pname}", bufs=2))
    ps = ctx.enter_context(tc.tile_pool(name=f"psp_{pname}", bufs=2, space="PSUM"))
    for p in range(NPAIR):
        wblk = wp.tile([P, DC, P], wdt, tag="wblk", name="wblk")
        nc.sync.dma_start(
            wblk[:], w_dram[:, P * p:P * (p + 1)].rearrange("(c q) m -> q c m", q=P)
        )
        pst = ps.tile([P, 1024], F32, tag="psproj", name="psproj")
        for c in range(2):
            for dc in range(DC):
                nc.tensor.matmul(
                    pst[:, 512 * c:512 * (c + 1)],
                    cast(wblk[:, dc, :]),
                    cast(src_T[:, dc, 512 * c:512 * (c + 1)]),
                    start=(dc == 0),
                    stop=(dc == DC - 1),
                )
        nc.vector.tensor_copy(dst[:, p, :], pst[:])


def _v_nat(ctx, tc, v_sb, src_T, w_dram, pname):
    """v_sb [128, SC, H, DH+1] bf16 = x @ Wv per head (natural), + ones col."""
    nc = tc.nc
    wv = ctx.enter_context(tc.tile_pool(name=f"wv_{pname}", bufs=1))
    ps = ctx.enter_context(tc.tile_pool(name=f"psv_{pname}", bufs=2, space="PSUM"))
    for half in range(2):
        wvt = wv.tile([P, DC, 512], F32R, tag="wvt", name="wvt")
        nc.sync.dma_start(
            wvt[:],
            w_dram[:, 512 * half:512 * (half + 1)].rearrange(
                "(c q) m -> q c m", q=P),
        )
        for jj in range(SC):
            pst = ps.tile([P, 512], F32, tag="psv", name="psv")
            for dc in range(DC):
                nc.tensor.matmul(
                    pst[:],
                    src_T[:, dc, P * jj:P * (jj + 1)],
                    wvt[:, dc, :],
                    start=(dc == 0),
                    stop=(dc == DC - 1),
                )
            nc.vector.tensor_copy(
                v_sb[:, jj, 8 * half:8 * (half + 1), 0:DH],
                pst.rearrange("p (h k) -> p h k", k=DH),
            )
    for jj in range(SC):
        nc.vector.memset(v_sb[:, jj, :, DH:DH + 1], 1.0)


def _attention(ctx, tc, heads_t, qt, kt, v_sb, masks, ident, causal, pname):
    """heads_t [128, DC, S] bf16 <- normalized per-head attention outputs."""
    nc = tc.nc
    ubufs = 6 if causal else 10
    up = ctx.enter_context(tc.tile_pool(name=f"u_{pname}", bufs=ubufs))
    pss = ctx.enter_context(tc.tile_pool(name=f"pss_{pname}", bufs=2, space="PSUM"))
    pso = ctx.enter_context(tc.tile_pool(name=f"pso_{pname}", bufs=2, space="PSUM"))
    rp = ctx.enter_context(tc.tile_pool(name=f"r_{pname}", bufs=3))
    rdp = ctx.enter_context(
        tc.tile_pool(name=f"rd_{pname}", bufs=3, space="DRAM"))
    for p in range(NPAIR):
        for h01 in range(2):
            head = 2 * p + h01
            base = DH * h01
            ot = pso.tile([DH + 1, 1024], F32, tag="pso", name="pso")
            if causal:
                for c in range(2):
                    nj = 4 * (c + 1)          # jj in [0, nj)
                    u_tiles = []
                    for pr in range(nj // 2):
                        pst = pss.tile([P, 1024], F32, tag="pss", name="pss")
                        for par in range(2):
                            jj = 2 * pr + par
                            t = jj - 4 * c
                            seg = pst[:, 512 * par:512 * (par + 1)]
                            nc.tensor.matmul(
                                seg,
                                kt[base:base + DH, p, P * jj:P * (jj + 1)],
                                qt[base:base + DH, p, 512 * c:512 * (c + 1)],
                                start=True,
                                stop=(t < 0),
                                tile_position=(base, 0),
                            )
                            if t >= 0:
                                nc.tensor.matmul(
                                    seg,
                                    ident[:],
                                    masks[:, t, :],
                                    start=False,
                                    stop=True,
                                    tile_position=(0, 0),
                                )
                        ut = up.tile([P, 1024], BF16, tag="u", name="ut")
                        nc.scalar.activation(ut[:], pst[:], AF.Exp, scale=0.125)
                        u_tiles.append(ut)
                    for jj in range(nj):
                        nc.tensor.matmul(
                            ot[:, 512 * c:512 * (c + 1)],
                            v_sb[:, jj, head, :],
                            u_tiles[jj // 2][:, 512 * (jj % 2):512 * (jj % 2 + 1)],
                            start=(jj == 0),
                            stop=(jj == nj - 1),
                        )
            else:
                u_tiles = []
                for jj in range(SC):
                    pst = pss.tile([P, 1024], F32, tag="pss", name="pss")
                    for c in range(2):
                        nc.tensor.matmul(
                            pst[:, 512 * c:512 * (c + 1)],
                            kt[base:base + DH, p, P * jj:P * (jj + 1)],
                            qt[base:base + DH, p, 512 * c:512 * (c + 1)],
                            start=True,
                            stop=True,
                            tile_position=(base, 0),
                        )
                    ut = up.tile([P, 1024], BF16, tag="u", name="ut")
                    nc.scalar.activation(ut[:], pst[:], AF.Exp, scale=0.125)
                    u_tiles.append(ut)
                for jj in range(SC):
                    for c in range(2):
                        nc.tensor.matmul(
                            ot[:, 512 * c:512 * (c + 1)],
                            v_sb[:, jj, head, :],
                            u_tiles[jj][:, 512 * c:512 * (c + 1)],
                            start=(jj == 0),
                            stop=(jj == SC - 1),
                        )
            # 1/denom via exp(-ln(d)) on ACT (same table set as softmax
            # exp); broadcast along partitions via a DRAM round-trip DMA.
            recip = rp.tile([1, 1024], F32, tag="recip", name="recip")
            nc.scalar.activation(recip[:], ot[DH:DH + 1, :], AF.Ln)
            nc.scalar.activation(recip[:], recip[:], AF.Exp, scale=-1.0)
            rdram = rdp.tile([1, 1024], F32, tag="rd", name="rdram")
            nc.sync.dma_start(rdram[:], recip[:])
            rb = rp.tile([DH, 1024], F32, tag="rb", name="rb")
            nc.sync.dma_start(rb[:], rdram.to_broadcast([DH, 1024]))
            nc.vector.tensor_tensor(
                heads_t[base:base + DH, p, :], ot[0:DH, :], rb[:], ALU.mult
            )


def _wo_res_ln(ctx, tc, x_out, heads_t, wo_dram, res_fn, spill_dram,
               stats, eps_t, pname, g_t=None, b_t=None):
    """x_out[:, s, :] = LN(heads @ WO + res).  Optionally spill to DRAM."""
    nc = tc.nc
    wo = ctx.enter_context(tc.tile_pool(name=f"wo_{pname}", bufs=1))
    ps = ctx.enter_context(tc.tile_pool(name=f"psa_{pname}", bufs=2, space="PSUM"))
    xr = ctx.enter_context(tc.tile_pool(name=f"xr_{pname}", bufs=3))
    wot = wo.tile([P, DC, 1024], BF16, name="wot")
    nc.sync.dma_start(wot[:], wo_dram.rearrange("(c q) m -> q c m", q=P))
    for s in range(SC):
        pst = ps.tile([P, 1024], F32, tag="psa", name="psa")
        for half in range(2):
            for dc in range(DC):
                nc.tensor.matmul(
                    pst[:, 512 * half:512 * (half + 1)],
                    heads_t[:, dc, P * s:P * (s + 1)],
                    wot[:, dc, 512 * half:512 * (half + 1)],
                    start=(dc == 0),
                    stop=(dc == DC - 1),
                )
        xrt = xr.tile([P, 1024], F32, tag="xrt", name="xrt")
        nc.sync.dma_start(xrt[:], res_fn(s))
        r_t = x_out[:, s, :]
        nc.vector.tensor_tensor(r_t, pst[:], xrt[:], ALU.add)
        _ln(nc, stats, r_t, r_t, eps_t, g_t, b_t)
        if spill_dram is not None:
            nc.sync.dma_start(spill_dram[:, s, :], r_t)


def _transpose_act(ctx, tc, dst, src, ident, pname):
    """dst [128, DC, S] = transpose of src [128, SC, D] (PE transpose)."""
    nc = tc.nc
    ps = ctx.enter_context(tc.tile_pool(name=f"pst_{pname}", bufs=2, space="PSUM"))
    for dc in range(DC):
        for sg in range(2):
            pt = ps.tile([P, 512], F32, tag="pst", name="pst")
            for s4 in range(4):
                s = 4 * sg + s4
                nc.tensor.transpose(
                    pt[:, P * s4:P * (s4 + 1)],
                    src[:, s, P * dc:P * (dc + 1)],
                    ident[:],
                )
            nc.vector.tensor_copy(dst[:, dc, 512 * sg:512 * (sg + 1)], pt[:])


def build(nc, trivial_ln=True, trivial_bias=True):
    """Emit the full per-core program onto nc."""
    def din(name, shape, dtype=F32):
        return nc.dram_tensor(name, list(shape), dtype, kind="ExternalInput").ap()

    xT_d = din("xT", (D, S), F32R)
    xnat_d = din("xnat", (S, D))
    eT_d = din("eT", (D, S), F32R)
    wq1_d = din("wq1t", (D, D), F32R)
    wk1_d = din("wk1t", (D, D), F32R)
    wv1_d = din("wv1t", (D, D), F32R)
    wo1_d = din("wo1t", (D, D), BF16)
    wq2_d = din("wq2t", (D, D), BF16)
    wk2_d = din("wk2t", (D, D), F32R)
    wv2_d = din("wv2t", (D, D), F32R)
    wo2_d = din("wo2t", (D, D), BF16)
    w1_d = din("w1t", (D, F), BF16)
    w2_d = din("w2t", (F, D), BF16)
    b1_d = din("b1r", (P, FC))
    masks_d = din("masks", (P, 4, 512), F32R)
    ident_d = din("ident", (P, P))
    identr_d = din("identr", (P, P), F32R)
    if not trivial_bias:
        b2_d = din("b2bc", (1, D))
    if not trivial_ln:
        lng_d = [din(f"ln{i}g", (1, D)) for i in (1, 2, 3)]
        lnb_d = [din(f"ln{i}b", (1, D)) for i in (1, 2, 3)]
    out_d = nc.dram_tensor("out", [S, D], F32, kind="ExternalOutput").ap()

    with ExitStack() as ctx:
        tc = ctx.enter_context(tile.TileContext(nc))
        consts = ctx.enter_context(tc.tile_pool(name="consts", bufs=1))
        stats = ctx.enter_context(tc.tile_pool(name="stats", bufs=4))
        dram = ctx.enter_context(tc.tile_pool(name="dram", bufs=1, space="DRAM"))
        bridge = ctx.enter_context(tc.tile_pool(name="bridge", bufs=1))

        ident = consts.tile([P, P], F32, name="ident")
        nc.sync.dma_start(ident[:], ident_d[:])
        identr = consts.tile([P, P], F32R, name="identr")
        nc.sync.dma_start(identr[:], identr_d[:])
        masks = consts.tile([P, 4, 512], F32R, name="masks")
        nc.sync.dma_start(masks[:], masks_d[:])
        b1r = consts.tile([P, FC], F32, name="b1r")
        nc.sync.dma_start(b1r[:], b1_d[:])
        eps_t = consts.tile([P, 1], F32, name="eps_t")
        nc.vector.memset(eps_t[:], EPS)
        g_ts = [None] * 3
        b_ts = [None] * 3
        if not trivial_ln:
            for i in range(3):
                g_ts[i] = consts.tile([P, D], F32, name=f"lng{i}")
                nc.sync.dma_start(g_ts[i][:], lng_d[i].to_broadcast([P, D]))
                b_ts[i] = consts.tile([P, D], F32, name=f"lnb{i}")
                nc.sync.dma_start(b_ts[i][:], lnb_d[i].to_broadcast([P, D]))
        b2bc = None
        if not trivial_bias:
            b2bc = consts.tile([P, D], F32, name="b2bc_t")
            nc.sync.dma_start(b2bc[:], b2_d.to_broadcast([P, D]))

        x1_spill = dram.tile([P, SC, D], F32, name="x1_spill")
        x2_spill = dram.tile([P, SC, D], F32, name="x2_spill")
        x1t = bridge.tile([P, DC, S], BF16, name="x1t")
        x2t = bridge.tile([P, DC, S], BF16, name="x2t")

        # ---------------- Phase 1: causal self-attention ----------------
        p_vqk = tc.alloc_tile_pool(name="p1vqk", bufs=1)
        v_sb = p_vqk.tile([P, SC, H, DH + 1], BF16, name="v_sb")
        qt = p_vqk.tile([P, NPAIR, S], F32R, name="qt")
        kt = p_vqk.tile([P, NPAIR, S], F32R, name="kt")
        with ExitStack() as seg:
            xt_pool = seg.enter_context(tc.tile_pool(name="p1xt", bufs=1))
            xt = xt_pool.tile([P, DC, S], F32R, name="xt")
            nc.sync.dma_start(xt[:], xT_d.rearrange("(c p) s -> p c s", p=P))
            with ExitStack() as s2:
                _v_nat(s2, tc, v_sb, xt, wv1_d, "v1")
            with ExitStack() as s2:
                _projection_T(s2, tc, qt, xt, wq1_d, "q1")
            with ExitStack() as s2:
                _projection_T(s2, tc, kt, xt, wk1_d, "k1")
        p_h = tc.alloc_tile_pool(name="p1h", bufs=1, side="right")
        heads_t = p_h.tile([P, DC, S], BF16, name="heads_t")
        with ExitStack() as seg:
            _attention(seg, tc, heads_t, qt, kt, v_sb, masks, identr,
                       True, "a1")
        p_vqk.release()
        with ExitStack() as seg:
            x1_pool = seg.enter_context(tc.tile_pool(name="p1x1", bufs=1))
            x1 = x1_pool.tile([P, SC, D], F32, name="x1")
            _wo_res_ln(seg, tc, x1, heads_t, wo1_d,
                       lambda s: xnat_d[P * s:P * (s + 1), :],
                       x1_spill, stats, eps_t, "o1", g_ts[0], b_ts[0])
            _transpose_act(seg, tc, x1t, x1, ident, "t1")
        p_h.release()

        # ---------------- Phase 2: cross-attention ----------------------
        p_vqk2 = tc.alloc_tile_pool(name="p2vqk", bufs=1)
        v2 = p_vqk2.tile([P, SC, H, DH + 1], BF16, name="v2")
        qt2 = p_vqk2.tile([P, NPAIR, S], F32R, name="qt2")
        kt2 = p_vqk2.tile([P, NPAIR, S], F32R, name="kt2")
        with ExitStack() as seg:
            _projection_T(seg, tc, qt2, x1t, wq2_d, "q2", bf=True)
        with ExitStack() as seg:
            et_pool = seg.enter_context(tc.tile_pool(name="p2et", bufs=1))
            et = et_pool.tile([P, DC, S], F32R, name="et")
            nc.sync.dma_start(et[:], eT_d.rearrange("(c p) s -> p c s", p=P))
            with ExitStack() as s2:
                _v_nat(s2, tc, v2, et, wv2_d, "v2")
            with ExitStack() as s2:
                _projection_T(s2, tc, kt2, et, wk2_d, "k2")
        p_h2 = tc.alloc_tile_pool(name="p2h", bufs=1, side="right")
        heads2 = p_h2.tile([P, DC, S], BF16, name="heads2")
        with ExitStack() as seg:
            _attention(seg, tc, heads2, qt2, kt2, v2, masks, identr,
                       False, "a2")
        p_vqk2.release()
        with ExitStack() as seg:
            x2_pool = seg.enter_context(tc.tile_pool(name="p2x2", bufs=1))
            x2 = x2_pool.tile([P, SC, D], F32, name="x2")
            _wo_res_ln(seg, tc, x2, heads2, wo2_d,
                       lambda s: x1_spill[:, s, :],
                       x2_spill, stats, eps_t, "o2", g_ts[1], b_ts[1])
            _transpose_act(seg, tc, x2t, x2, ident, "t2")
        p_h2.release()

        # ---------------- Phase 3: FFN ----------------------------------
        with ExitStack() as ph3:
            a3 = ph3.enter_context(tc.tile_pool(name="p3acts", bufs=1))
            w1p = ph3.enter_context(tc.tile_pool(name="w1p", bufs=3))
            w2p = ph3.enter_context(tc.tile_pool(name="w2p", bufs=1))
            psh = ph3.enter_context(tc.tile_pool(name="psh", bufs=2, space="PSUM"))
            psy = ph3.enter_context(tc.tile_pool(name="psy", bufs=2, space="PSUM"))
            xr2 = ph3.enter_context(tc.tile_pool(name="xr2", bufs=3))
            y_sb = a3.tile([P, SC, D], F32, name="y_sb")
            for fh in range(2):
                ht = a3.tile([P, 16, S], BF16, tag="ht", name="ht")
                w2c = w2p.tile([P, 16, D], BF16, tag="w2c", name="w2c")
                nc.sync.dma_start(
                    w2c[:],
                    w2_d[2048 * fh:2048 * (fh + 1), :].rearrange(
                        "(f q) d -> q f d", q=P),
                )
                for ff in range(16):
                    ft = 16 * fh + ff
                    w1b = w1p.tile([P, DC, P], BF16, tag="w1b", name="w1b")
                    nc.sync.dma_start(
                        w1b[:],
                        w1_d[:, P * ft:P * (ft + 1)].rearrange(
                            "(c q) m -> q c m", q=P),
                    )
                    ph = psh.tile([P, S], F32, tag="ph", name="ph")
                    for half in range(2):
                        for dc in range(DC):
                            nc.tensor.matmul(
                                ph[:, 512 * half:512 * (half + 1)],
                                w1b[:, dc, :],
                                x2t[:, dc, 512 * half:512 * (half + 1)],
                                start=(dc == 0),
                                stop=(dc == DC - 1),
                            )
                    nc.scalar.activation(
                        ht[:, ff, :], ph[:], AF.Gelu, bias=b1r[:, ft:ft + 1]
                    )
                for s in range(SC):
                    py = psy.tile([P, D], F32, tag="py", name="py")
                    for half in range(2):
                        for ff in range(16):
                            nc.tensor.matmul(
                                py[:, 512 * half:512 * (half + 1)],
                                ht[:, ff, P * s:P * (s + 1)],
                                w2c[:, ff, 512 * half:512 * (half + 1)],
                                start=(ff == 0),
                                stop=(ff == 15),
                            )
                    if fh == 0:
                        xrt = xr2.tile([P, D], F32, tag="xr2t", name="xrt2")
                        nc.sync.dma_start(xrt[:], x2_spill[:, s, :])
                        nc.vector.tensor_tensor(
                            y_sb[:, s, :], py[:], xrt[:], ALU.add)
                    else:
                        nc.vector.tensor_tensor(
                            y_sb[:, s, :], py[:], y_sb[:, s, :], ALU.add)
            for s in range(SC):
                r_t = y_sb[:, s, :]
                if b2bc is not None:
                    nc.vector.tensor_tensor(r_t, r_t, b2bc, ALU.add)
                _ln(nc, stats, r_t, r_t, eps_t, g_ts[2], b_ts[2])
                nc.sync.dma_start(out_d[P * s:P * (s + 1), :], r_t)


def make_host_inputs(inputs):
    """Host-side prep: transposes, packing, masks. Returns (common, percore)."""
    import ml_dtypes
    f32 = np.float32
    bf16 = ml_dtypes.bfloat16

    def packT(w):  # [H, D, DH] -> [D, H*DH] (W^T with head-major cols)
        return np.ascontiguousarray(
            np.asarray(w, f32).transpose(1, 0, 2).reshape(D, D))

    rows = np.arange(P)[:, None]
    cols = np.arange(512)[None, :]
    masks = np.stack(
        [np.where(cols >= P * t + rows, 0.0, NEG).astype(f32) for t in range(4)],
        axis=1,
    )  # [128, 4, 512]

    common = {
        "wq1t": packT(inputs["wq1"]),
        "wk1t": packT(inputs["wk1"]),
        "wv1t": packT(inputs["wv1"]),
        "wo1t": np.ascontiguousarray(np.asarray(inputs["wo1"], f32).T).astype(bf16),
        "wq2t": packT(inputs["wq2"]).astype(bf16),
        "wk2t": packT(inputs["wk2"]),
        "wv2t": packT(inputs["wv2"]),
        "wo2t": np.ascontiguousarray(np.asarray(inputs["wo2"], f32).T).astype(bf16),
        "w1t": np.ascontiguousarray(np.asarray(inputs["w1"], f32).T).astype(bf16),
        "w2t": np.ascontiguousarray(np.asarray(inputs["w2"], f32).T).astype(bf16),
        "b1r": np.ascontiguousarray(np.asarray(inputs["b1"], f32).reshape(FC, P).T),
        "masks": masks,
        "ident": np.eye(P, dtype=f32),
        "identr": np.eye(P, dtype=f32),
    }
    trivial_ln = all(
        np.all(np.asarray(inputs[f"ln{i}_g"]) == 1.0)
        and np.all(np.asarray(inputs[f"ln{i}_b"]) == 0.0)
        for i in (1, 2, 3)
    )
    trivial_bias = bool(np.all(np.asarray(inputs["b2"]) == 0.0))
    if not trivial_ln:
        for i in (1, 2, 3):
            common[f"ln{i}g"] = np.asarray(inputs[f"ln{i}_g"], f32).reshape(1, D)
            common[f"ln{i}b"] = np.asarray(inputs[f"ln{i}_b"], f32).reshape(1, D)
    if not trivial_bias:
        common["b2bc"] = np.asarray(inputs["b2"], f32).reshape(1, D)

    emb = np.asarray(inputs["embeds"], f32)
    enc = np.asarray(inputs["encoder_output"], f32)
    percore = []
    for b in range(B):
        percore.append({
            "xT": np.ascontiguousarray(emb[b].T),
            "xnat": np.ascontiguousarray(emb[b]),
            "eT": np.ascontiguousarray(enc[b].T),
        })
    return common, percore, trivial_ln, trivial_bias


def build_module(inputs):
    common, percore, trivial_ln, trivial_bias = make_host_inputs(inputs)
    nc = bacc.Bacc("TRN2", target_bir_lowering=False, debug=False,
                   enable_asserts=False, num_devices=B)
    build(nc, trivial_ln=trivial_ln, trivial_bias=trivial_bias)
    nc.compile()
    in_maps = [{**common, **pc} for pc in percore]
    return nc, in_maps


def kernel(**inputs) -> np.ndarray:
    nc, in_maps = build_module(inputs)
    res = run_bass_kernel_spmd(nc, in_maps, core_ids=list(range(B)))
    return np.stack([r["out"] for r in res.results]).astype(np.float32)
